# revision 5
# baseline (speedup 1.0000x reference)
"""Trainium2 Bass kernel for the label-selected log-softmax loss.

Math: per sample with logits [s, a] and label l in {0,1,2}:
    lp = log_softmax([s, a]);  err = (l==1)?lp[0] : (l==2)?lp[1] : 0
    loss = -mean(err)
With d = s - a:
    lp[0] = -softplus(-d),  lp[1] = -softplus(d)
    loss  = (1/B) * sum over l!=0 of softplus(c*d),  c = -1 if l==1, +1 if l==2
On device we compute sum over ALL samples of softplus(c*d) with c=0 for l==0
(which contributes softplus(0)=ln2 each); the host subtracts N0*ln2 in the
final unshard step.  Data parallel over 8 cores; each core reduces its shard
to a [128,1] per-partition partial that the host combines.
"""

import sys

sys.path.insert(0, "/opt/trn_rl_repo")

import numpy as np
import concourse.bass as bass
import concourse.bacc as bacc
import concourse.mybir as mybir
from concourse.tile import TileContext
from concourse.bass_utils import run_bass_kernel_spmd

N_CORES = 8
B = 8388608
NC = B // N_CORES  # 1048576 samples per core
P = 128
FTOT = NC // P  # 8192 free elements per partition
F = 1024  # tile free-dim
NT = FTOT // F

LN2 = float(np.log(2.0))

_cache = {}
last_result = None  # BassKernelResults of the most recent run (for profiling)


def _build():
    if "nc" in _cache:
        return _cache["nc"]
    nc = bacc.Bacc()
    s_d = nc.declare_dram_parameter("s", [P, FTOT], mybir.dt.float32, isOutput=False)
    a_d = nc.declare_dram_parameter("a", [P, FTOT], mybir.dt.float32, isOutput=False)
    c_d = nc.declare_dram_parameter("c", [P, FTOT], mybir.dt.int8, isOutput=False)
    out_d = nc.declare_dram_parameter("partial", [P, 1], mybir.dt.float32, isOutput=True)

    f32 = mybir.dt.float32
    with TileContext(nc) as tc:
        with tc.tile_pool(name="io", bufs=4) as io, tc.tile_pool(name="accp", bufs=1) as accp:
            acc = accp.tile([P, NT], f32, tag="acc")
            for i in range(NT):
                s_t = io.tile([P, F], f32, tag="s")
                a_t = io.tile([P, F], f32, tag="a")
                c_t = io.tile([P, F], mybir.dt.int8, tag="c")
                nc.gpsimd.dma_start(out=s_t[:], in_=s_d[:, i * F : (i + 1) * F])
                nc.gpsimd.dma_start(out=a_t[:], in_=a_d[:, i * F : (i + 1) * F])
                nc.gpsimd.dma_start(out=c_t[:], in_=c_d[:, i * F : (i + 1) * F])
                # All compute in-place on s_t: each instruction then needs at
                # most 2 semaphore waits (the HW per-instruction limit that
                # "Too many sync wait commands" enforces).
                nc.vector.tensor_sub(s_t[:], s_t[:], a_t[:])
                nc.vector.tensor_mul(s_t[:], s_t[:], c_t[:])
                # softplus(z) = ln(exp(z) + 1); Softplus itself is not in the
                # compiler's ACT function tables, but exp+ln share a set.
                nc.scalar.activation(s_t[:], s_t[:], mybir.ActivationFunctionType.Exp)
                nc.scalar.activation(
                    s_t[:],
                    s_t[:],
                    mybir.ActivationFunctionType.Ln,
                    bias=1.0,
                    accum_out=acc[:, i : i + 1],
                )
            col = accp.tile([P, 1], f32, tag="col")
            nc.vector.reduce_sum(col[:], acc[:], axis=mybir.AxisListType.X)
            nc.gpsimd.dma_start(out=out_d[:], in_=col[:])
    nc.compile()
    _cache["nc"] = nc
    return nc


def kernel(synonymy_score, antonymy_score, labels):
    global last_result
    s = np.ascontiguousarray(np.asarray(synonymy_score, dtype=np.float32).reshape(-1))
    a = np.ascontiguousarray(np.asarray(antonymy_score, dtype=np.float32).reshape(-1))
    lab = np.asarray(labels).reshape(-1)
    c = (lab == 2).astype(np.int8) - (lab == 1).astype(np.int8)
    n0 = int(np.count_nonzero(lab == 0))

    nc = _build()
    in_maps = []
    for k in range(N_CORES):
        sl = slice(k * NC, (k + 1) * NC)
        in_maps.append(
            {
                "s": s[sl].reshape(P, FTOT),
                "a": a[sl].reshape(P, FTOT),
                "c": np.ascontiguousarray(c[sl]).reshape(P, FTOT),
            }
        )
    res = run_bass_kernel_spmd(nc, in_maps, list(range(N_CORES)))
    last_result = res
    total = 0.0
    for r in res.results:
        total += float(np.asarray(r["partial"], dtype=np.float64).sum())
    loss = (total - n0 * LN2) / B
    return np.float32(loss)


# revision 6
# speedup vs baseline: 1.2487x; 1.2487x over previous
"""Trainium2 Bass kernel for the label-selected log-softmax loss.

Math: per sample with logits [s, a] and label l in {0,1,2}:
    lp = log_softmax([s, a]);  err = (l==1)?lp[0] : (l==2)?lp[1] : 0
    loss = -mean(err)
With d = s - a:
    lp[0] = -softplus(-d),  lp[1] = -softplus(d)
    loss  = (1/B) * sum over l!=0 of softplus(c*d),  c = -1 if l==1, +1 if l==2
On device we compute sum over ALL samples of softplus(c*d) with c=0 for l==0
(which contributes softplus(0)=ln2 each); the host subtracts N0*ln2 in the
final unshard step.  Data parallel over 8 cores; each core reduces its shard
to a [128,1] per-partition partial that the host combines.
"""

import sys

sys.path.insert(0, "/opt/trn_rl_repo")

import numpy as np
import concourse.bass as bass
import concourse.bacc as bacc
import concourse.mybir as mybir
from concourse.tile import TileContext
from concourse.bass_utils import run_bass_kernel_spmd

N_CORES = 8
B = 8388608
NC = B // N_CORES  # 1048576 samples per core
P = 128
FTOT = NC // P  # 8192 free elements per partition
F = 1024  # tile free-dim
NT = FTOT // F

LN2 = float(np.log(2.0))

_cache = {}
last_result = None  # BassKernelResults of the most recent run (for profiling)


def _build():
    if "nc" in _cache:
        return _cache["nc"]
    nc = bacc.Bacc()
    s_d = nc.declare_dram_parameter("s", [P, FTOT], mybir.dt.float32, isOutput=False)
    a_d = nc.declare_dram_parameter("a", [P, FTOT], mybir.dt.float32, isOutput=False)
    c_d = nc.declare_dram_parameter("c", [P, FTOT], mybir.dt.int8, isOutput=False)
    out_d = nc.declare_dram_parameter("partial", [P, 1], mybir.dt.float32, isOutput=True)

    f32 = mybir.dt.float32
    CH = 4  # tiles per chunk; one wide Ln per chunk keeps ACT table reloads rare
    NCHUNK = NT // CH
    with TileContext(nc) as tc:
        with tc.tile_pool(name="io", bufs=6) as io, tc.tile_pool(name="zp", bufs=1) as zp:
            z_all = zp.tile([P, FTOT], f32, tag="z")
            acc = zp.tile([P, NCHUNK], f32, tag="acc")
            for ci in range(NCHUNK):
                for j in range(CH):
                    i = ci * CH + j
                    s_t = io.tile([P, F], f32, tag="s")
                    a_t = io.tile([P, F], f32, tag="a")
                    c_t = io.tile([P, F], mybir.dt.int8, tag="c")
                    nc.sync.dma_start(out=s_t[:], in_=s_d[:, i * F : (i + 1) * F])
                    nc.sync.dma_start(out=a_t[:], in_=a_d[:, i * F : (i + 1) * F])
                    nc.sync.dma_start(out=c_t[:], in_=c_d[:, i * F : (i + 1) * F])
                    zi = z_all[:, i * F : (i + 1) * F]
                    nc.vector.tensor_sub(zi, s_t[:], a_t[:])
                    nc.vector.tensor_mul(zi, zi, c_t[:])
                    # softplus(z) = ln(exp(z) + 1); Softplus itself is not in
                    # the compiler's ACT function tables, but exp+ln share one.
                    nc.scalar.activation(zi, zi, mybir.ActivationFunctionType.Exp)
                zc = z_all[:, ci * CH * F : (ci + 1) * CH * F]
                nc.scalar.activation(
                    zc,
                    zc,
                    mybir.ActivationFunctionType.Ln,
                    bias=1.0,
                    accum_out=acc[:, ci : ci + 1],
                )
            col = zp.tile([P, 1], f32, tag="col")
            nc.vector.reduce_sum(col[:], acc[:], axis=mybir.AxisListType.X)
            nc.gpsimd.dma_start(out=out_d[:], in_=col[:])
    nc.compile()
    _cache["nc"] = nc
    return nc


def kernel(synonymy_score, antonymy_score, labels):
    global last_result
    s = np.ascontiguousarray(np.asarray(synonymy_score, dtype=np.float32).reshape(-1))
    a = np.ascontiguousarray(np.asarray(antonymy_score, dtype=np.float32).reshape(-1))
    lab = np.asarray(labels).reshape(-1)
    c = (lab == 2).astype(np.int8) - (lab == 1).astype(np.int8)
    n0 = int(np.count_nonzero(lab == 0))

    nc = _build()
    in_maps = []
    for k in range(N_CORES):
        sl = slice(k * NC, (k + 1) * NC)
        in_maps.append(
            {
                "s": s[sl].reshape(P, FTOT),
                "a": a[sl].reshape(P, FTOT),
                "c": np.ascontiguousarray(c[sl]).reshape(P, FTOT),
            }
        )
    res = run_bass_kernel_spmd(nc, in_maps, list(range(N_CORES)))
    last_result = res
    total = 0.0
    for r in res.results:
        total += float(np.asarray(r["partial"], dtype=np.float64).sum())
    loss = (total - n0 * LN2) / B
    return np.float32(loss)


# revision 7
# speedup vs baseline: 1.2628x; 1.0112x over previous
"""Trainium2 Bass kernel for the label-selected log-softmax loss.

Math: per sample with logits [s, a] and label l in {0,1,2}:
    lp = log_softmax([s, a]);  err = (l==1)?lp[0] : (l==2)?lp[1] : 0
    loss = -mean(err)
With d = s - a:
    lp[0] = -softplus(-d),  lp[1] = -softplus(d)
    loss  = (1/B) * sum over l!=0 of softplus(c*d),  c = -1 if l==1, +1 if l==2
On device we compute sum over ALL samples of softplus(c*d) with c=0 for l==0
(which contributes softplus(0)=ln2 each); the host subtracts N0*ln2 in the
final unshard step.  Data parallel over 8 cores; each core reduces its shard
to a [128,1] per-partition partial that the host combines.
"""

import sys

sys.path.insert(0, "/opt/trn_rl_repo")

import numpy as np
import concourse.bass as bass
import concourse.bacc as bacc
import concourse.mybir as mybir
from concourse.tile import TileContext
from concourse.bass_utils import run_bass_kernel_spmd

N_CORES = 8
B = 8388608
NC = B // N_CORES  # 1048576 samples per core
P = 128
FTOT = NC // P  # 8192 free elements per partition
F = 1024  # tile free-dim
NT = FTOT // F

LN2 = float(np.log(2.0))

_cache = {}
last_result = None  # BassKernelResults of the most recent run (for profiling)


def _build():
    if "nc" in _cache:
        return _cache["nc"]
    nc = bacc.Bacc()
    sa_d = nc.declare_dram_parameter("sa", [P, 2 * FTOT], mybir.dt.float32, isOutput=False)
    c_d = nc.declare_dram_parameter("c", [P, FTOT], mybir.dt.int8, isOutput=False)
    out_d = nc.declare_dram_parameter("partial", [P, 1], mybir.dt.float32, isOutput=True)

    f32 = mybir.dt.float32
    CH = 4  # tiles per chunk; one wide Ln per chunk keeps ACT table reloads rare
    NCHUNK = NT // CH
    with TileContext(nc) as tc:
        with tc.tile_pool(name="io", bufs=6) as io, tc.tile_pool(name="zp", bufs=1) as zp:
            z_all = zp.tile([P, FTOT], f32, tag="z")
            acc = zp.tile([P, NCHUNK], f32, tag="acc")
            for ci in range(NCHUNK):
                for j in range(CH):
                    i = ci * CH + j
                    sa_t = io.tile([P, 2 * F], f32, tag="sa")
                    c_t = io.tile([P, F], mybir.dt.int8, tag="c")
                    nc.sync.dma_start(out=sa_t[:], in_=sa_d[:, i * 2 * F : (i + 1) * 2 * F])
                    nc.scalar.dma_start(out=c_t[:], in_=c_d[:, i * F : (i + 1) * F])
                    zi = z_all[:, i * F : (i + 1) * F]
                    nc.vector.tensor_sub(zi, sa_t[:, :F], sa_t[:, F : 2 * F])
                    nc.vector.tensor_mul(zi, zi, c_t[:])
                    # softplus(z) = ln(exp(z) + 1); Softplus itself is not in
                    # the compiler's ACT function tables, but exp+ln share one.
                    nc.scalar.activation(zi, zi, mybir.ActivationFunctionType.Exp)
                zc = z_all[:, ci * CH * F : (ci + 1) * CH * F]
                nc.scalar.activation(
                    zc,
                    zc,
                    mybir.ActivationFunctionType.Ln,
                    bias=1.0,
                    accum_out=acc[:, ci : ci + 1],
                )
            col = zp.tile([P, 1], f32, tag="col")
            nc.vector.reduce_sum(col[:], acc[:], axis=mybir.AxisListType.X)
            nc.gpsimd.dma_start(out=out_d[:], in_=col[:])
    nc.compile()
    _cache["nc"] = nc
    return nc


def kernel(synonymy_score, antonymy_score, labels):
    global last_result
    s = np.ascontiguousarray(np.asarray(synonymy_score, dtype=np.float32).reshape(-1))
    a = np.ascontiguousarray(np.asarray(antonymy_score, dtype=np.float32).reshape(-1))
    lab = np.asarray(labels).reshape(-1)
    c = (lab == 2).astype(np.int8) - (lab == 1).astype(np.int8)
    n0 = int(np.count_nonzero(lab == 0))

    nc = _build()
    in_maps = []
    for k in range(N_CORES):
        sl = slice(k * NC, (k + 1) * NC)
        # Interleave s and a at tile granularity: tile i occupies columns
        # [2iF, 2(i+1)F) with the s-chunk first, then the a-chunk, so one DMA
        # feeds both operands of the subtract.
        sa = np.empty((P, 2 * FTOT), dtype=np.float32)
        sa3 = sa.reshape(P, FTOT // F, 2 * F)
        sa3[:, :, :F] = s[sl].reshape(P, FTOT // F, F)
        sa3[:, :, F:] = a[sl].reshape(P, FTOT // F, F)
        in_maps.append(
            {
                "sa": sa,
                "c": np.ascontiguousarray(c[sl]).reshape(P, FTOT),
            }
        )
    res = run_bass_kernel_spmd(nc, in_maps, list(range(N_CORES)))
    last_result = res
    total = 0.0
    for r in res.results:
        total += float(np.asarray(r["partial"], dtype=np.float64).sum())
    loss = (total - n0 * LN2) / B
    return np.float32(loss)
